# revision 70
# baseline (speedup 1.0000x reference)
"""Trainium2 Bass kernel for nn_Attention_47768626266365.

Dense transformer block: ChanLayerNorm -> 1x1 conv qkv -> depthwise 3x3 convs
-> 8-head attention with relative-position bias -> 1x1 conv out.

Sharding: data-parallel over batch, 2 images per core across 8 cores.

Key structure (v2, tuned for HAM clock-gate warmth + engine balance):
  * LayerNorm folded into the qkv projection:
      qkv = rstd(t) * (W_eff @ x  -  mu(t) * rowsum(W_eff))
    The mean term rides the PSUM accumulation as a K=1 matmul with lhsT =
    -rowsum(W_eff); the rstd multiply is fused into the PSUM evacuation
    (DVE tensor_tensor) that also writes the zero-padded 34x34 layout.
    rstd = exp(-0.5 * ln(var + eps)) keeps every scalar-engine function in
    the natural_log_exp activation table (no table reloads).
  * depthwise 3x3: 9 taps split across engines -
      3 on PE (diag-matmul accumulated into the same PSUM tile),
      2 on scalar (per-partition-scale Copy),
      1 STT + 2 merge TTs on DVE, 2 STT on GPSIMD (chained).
  * attention per (head, j-chunk, img): simT on PE, exp on scalar,
    *exp(bias) on DVE, [v|1]-augmented AV accumulation on PE.
  * softmax denominators batched: per-image (8, TOK) fp32 tile, one
    reciprocal_approx_fast, K=8 selector matmul broadcasts.
  * two images software-pipelined: S1(img0) | S2(img0)+S1(img1) |
    S2(img1)+S3(img0) | S3(img1), keeping the PE instruction stream dense
    so the HAM clock gate stays at full rate.
"""

import os
import sys

sys.path.insert(0, "/opt/trn_rl_repo")

import numpy as np
from contextlib import ExitStack

import concourse.bass as bass
import concourse.bacc as bacc
import concourse.mybir as mybir
import concourse.tile as tile
from concourse.bass_utils import run_bass_kernel_spmd


F32 = mybir.dt.float32
F16 = mybir.dt.float16
AF = mybir.ActivationFunctionType
OP = mybir.AluOpType

# ---- problem constants (hardcoded per contract) ----
B, C, S = 16, 512, 32
TOK = S * S                     # 1024 tokens
HEADS, D = 8, 64
INNER = HEADS * D               # 512
O3 = 3 * INNER                  # 1536 qkv channels
NCORES = 8
IPC = B // NCORES               # images per core = 2
P = 128
PW = S + 2                      # padded row width 34
PTOK = PW * PW + 2              # 1156 + slack
EPS = 1e-5
SCALE = D ** -0.5
NOC = O3 // P                   # 12 qkv channel chunks
NCC = C // P                    # 4 input channel chunks
NJC = TOK // P                  # 8 token chunks

# canonical tap order for host-side dwW packing
TAPS = [(dx, dy) for dx in (-1, 0, 1) for dy in (-1, 0, 1)]
# dwconv tap placement is asymmetric per image: img0's stage-1 runs in
# phase A (scalar engine idle -> cheap Copy-with-scale taps); img1's
# stage-1 overlaps the img0 attention stream, whose latency path runs
# through the scalar engine's exp -- so img1 avoids scalar entirely.
PE_TAPS_UNION = list(TAPS)
TAP_CFG = {
    0: dict(pe=[(-1, 0), (0, 0), (1, 0)],
            sc=[(-1, -1), (1, 1), (1, -1), (-1, 1)],
            ch=[(0, -1), (0, 1)],
            mg=["v", "v", "v", "g"]),
    1: dict(pe=[(-1, 0), (0, 0), (1, 0), (0, -1), (0, 1), (1, -1)],
            sc=[],
            ch=[(-1, -1), (-1, 1), (1, 1)],
            mg=[]),
}
OC_ORDER = [0, 4, 8, 1, 5, 9, 2, 6, 10, 3, 7, 11]


def _tidx(tap):
    return TAPS.index(tap)


def _pad_view(t, off, rows):
    """(128, rows, 32) view into padded (128, PTOK) tile at element offset."""
    return t[:, off: off + rows * PW].rearrange("p (x y) -> p x y", y=PW)[:, :, :S]


def _tap_off(dx, dy):
    return (1 + dx) * PW + (1 + dy)


def build_nc():
    nc = bacc.Bacc("TRN2", target_bir_lowering=False, debug=False)

    x_d = nc.dram_tensor("x", (IPC, C, TOK), F32, kind="ExternalInput")
    wqkvT_d = nc.dram_tensor("wqkvT", (P, NCC, O3), F16, kind="ExternalInput")
    negwsum_d = nc.dram_tensor("negwsum", (1, O3), F16, kind="ExternalInput")
    woutT_d = nc.dram_tensor("woutT", (P, NCC, INNER), F16, kind="ExternalInput")
    dwW_d = nc.dram_tensor("dwW", (P, NOC, 9), F32, kind="ExternalInput")
    dwdiag_d = nc.dram_tensor(
        "dwdiag", (P, NOC, len(PE_TAPS_UNION), P), F16, kind="ExternalInput")
    ebt_d = nc.dram_tensor("ebt", (HEADS, NJC, P, TOK), F16, kind="ExternalInput")
    selpair_d = nc.dram_tensor("selpair", (2, P), F32, kind="ExternalInput")
    out_d = nc.dram_tensor("out", (IPC, C, TOK), F32, kind="ExternalOutput")
    DBG = bool(int(os.environ.get("KDBG", "0")))
    if DBG:
        dbgqk_d = nc.dram_tensor("dbgqk", (IPC, P, 8, TOK), F16,
                                 kind="ExternalOutput")
        dbgvh_d = nc.dram_tensor("dbgvh", (IPC, P, NJC, HEADS, 65), F16,
                                 kind="ExternalOutput")
        dbgot_d = nc.dram_tensor("dbgot", (IPC, P, NCC, TOK), F16,
                                 kind="ExternalOutput")
        dbgrc_d = nc.dram_tensor("dbgrc", (IPC, HEADS, TOK), F16,
                                 kind="ExternalOutput")
        dbgrs_d = nc.dram_tensor("dbgrs", (IPC, P, TOK), F16,
                                 kind="ExternalOutput")
        dbgdn_d = nc.dram_tensor("dbgdn", (IPC, HEADS, TOK), F32,
                                 kind="ExternalOutput")

    with tile.TileContext(nc) as tc, ExitStack() as ctx:
        const = ctx.enter_context(tc.tile_pool(name="const", bufs=1))
        persist = ctx.enter_context(tc.tile_pool(name="persist", bufs=1))
        qp = ctx.enter_context(tc.tile_pool(name="qp", bufs=4))
        accp = ctx.enter_context(tc.tile_pool(name="accp", bufs=6))
        dwp = ctx.enter_context(tc.tile_pool(name="dwp", bufs=2))
        ttp = ctx.enter_context(tc.tile_pool(name="ttp", bufs=4))
        ep = ctx.enter_context(tc.tile_pool(name="ep", bufs=4))
        Ep = ctx.enter_context(tc.tile_pool(name="Ep", bufs=3))
        ofp = ctx.enter_context(tc.tile_pool(name="ofp", bufs=2))
        small = ctx.enter_context(tc.tile_pool(name="small", bufs=1))
        # PSUM is phase-scoped: stage-1 pools live through phases A+B, then
        # close so phase C can double-buffer the attention PSUM.
        avp = ctx.enter_context(tc.tile_pool(name="avp", bufs=1, space="PSUM"))
        ab_ctx = ExitStack()
        mm = ab_ctx.enter_context(tc.tile_pool(name="mm", bufs=2, space="PSUM"))
        simB = ab_ctx.enter_context(
            tc.tile_pool(name="simB", bufs=1, space="PSUM"))

        # ---------- constants ----------
        wqkvT = const.tile([P, NCC, O3], F16, tag="wqkvT")
        nc.sync.dma_start(wqkvT[:], wqkvT_d[:])
        negwsum = const.tile([1, O3], F16, tag="negwsum")
        nc.sync.dma_start(negwsum[:], negwsum_d[:])
        woutT = const.tile([P, NCC, INNER], F16, tag="woutT")
        nc.sync.dma_start(woutT[:], woutT_d[:])
        dwW = const.tile([P, NOC, 9], F32, tag="dwW")
        nc.sync.dma_start(dwW[:], dwW_d[:])
        dwdiag = const.tile([P, NOC, len(PE_TAPS_UNION), P], F16, tag="dwdiag")
        nc.sync.dma_start(dwdiag[:], dwdiag_d[:])
        selA = const.tile([1, P], F32, tag="selA")
        nc.sync.dma_start(selA[:], selpair_d[0:1, :])
        selB = const.tile([1, P], F32, tag="selB")
        nc.sync.dma_start(selB[:], selpair_d[1:2, :])
        F32R = mybir.dt.float32r
        ones128 = const.tile([P, 1], F16, tag="ones128")
        nc.gpsimd.memset(ones128[:], 1.0)
        onesrow = const.tile([1, P], F16, tag="onesrow")
        nc.gpsimd.memset(onesrow[:], 1.0)
        epsc = const.tile([1, 1], F32, tag="epsc")
        nc.gpsimd.memset(epsc[:], EPS)
        zconst = const.tile([P, 1], F32, tag="zconst")
        nc.gpsimd.memset(zconst[:], 0.0)
        nc.const_aps.aps[(F32, 0.0)] = zconst[:]

        # ---------- per-image persistent tiles ----------
        xb = [persist.tile([P, NCC, TOK], F16, tag=f"xb{i}", name=f"xb{i}")
              for i in range(IPC)]
        qk_sb = [persist.tile([P, 8, TOK], F16, tag=f"qk{i}", name=f"qk{i}")
                 for i in range(IPC)]
        vhat = [persist.tile([P, NJC, HEADS, 65], F16, tag=f"vh{i}", name=f"vh{i}")
                for i in range(IPC)]
        outT = [persist.tile([P, NCC, TOK], F16, tag=f"ot{i}", name=f"ot{i}")
                for i in range(IPC)]
        rsbc = [persist.tile([P, TOK], F16, tag=f"rs{i}", name=f"rs{i}")
                for i in range(IPC)]
        mu16 = [persist.tile([1, TOK], F16, tag=f"mu{i}", name=f"mu{i}")
                for i in range(IPC)]
        rcp = ctx.enter_context(tc.tile_pool(name="rcp", bufs=2))
        rcd = ctx.enter_context(tc.tile_pool(name="rcd", bufs=1))

        # ones column of the [v | 1] augmented AV operand
        for i in range(IPC):
            nc.vector.memset(vhat[i][:, :, :, 64:65], 1.0)

        # x loads for both images up front (swdge queues, off engines)
        for img in range(IPC):
            for ci in range(NCC):
                nc.gpsimd.dma_start(xb[img][:, ci, :],
                                    x_d[img, ci * P:(ci + 1) * P, :])

        # ================= stage 1 generator =================
        def s1(img, late_pool=None):
            # --- LN stats (sequential per-half so the 1-buf mm pool works) ---
            sq = []
            for ci in range(NCC):
                xsq = qp.tile([P, TOK], F16, tag="xsq", name=f"xsq{img}_{ci}")
                nc.scalar.activation(xsq[:], xb[img][:, ci, :], AF.Square)
                sq.append(xsq)
            sc1 = small.tile([1, TOK], F32, tag="sc1", name=f"sc1{img}")
            for hf in range(2):
                sl = slice(hf * 512, (hf + 1) * 512)
                st = mm.tile([P, 512], F32, tag="mm", name=f"st{img}_{hf}")
                for ci in range(NCC):
                    nc.tensor.matmul(st[0:1, :], lhsT=ones128[:],
                                     rhs=xb[img][:, ci, sl],
                                     start=(ci == 0), stop=(ci == NCC - 1))
                    nc.tensor.matmul(st[32:33, :], lhsT=ones128[:],
                                     rhs=sq[ci][:, sl],
                                     start=(ci == 0), stop=(ci == NCC - 1))
                nc.vector.tensor_scalar(mu16[img][0:1, sl], st[0:1, :],
                                        1.0 / C, None, OP.mult)
                nc.vector.tensor_tensor(sc1[0:1, sl], mu16[img][0:1, sl],
                                        mu16[img][0:1, sl], OP.mult)
                nc.vector.scalar_tensor_tensor(
                    sc1[0:1, sl], st[32:33, :], 1.0 / C, sc1[0:1, sl],
                    OP.mult, OP.subtract)
            # rstd = exp(-0.5 * ln(var + eps)); stays in the exp/ln table
            nc.scalar.activation(sc1[:], sc1[:], AF.Ln, bias=epsc[0:1, :])
            rs16 = small.tile([1, TOK], F16, tag="rs16", name=f"rs16{img}")
            nc.scalar.activation(rs16[:], sc1[:], AF.Exp, scale=-0.5)
            # broadcast rstd across partitions via K=1 matmul
            bc = mm.tile([P, TOK], F32, tag="mm", name=f"bc{img}")
            for hf in range(2):
                sl = slice(hf * 512, (hf + 1) * 512)
                nc.tensor.matmul(bc[:, sl], lhsT=onesrow[:],
                                 rhs=rs16[0:1, sl], start=True, stop=True)
            nc.scalar.activation(rsbc[img][:], bc[:], AF.Copy)
            yield

            # --- qkv projection + dwconv per oc chunk ---
            for oc_i, oc in enumerate(OC_ORDER):
                # the tail oc-triples of img1 are emitted in phase C, after
                # the stage-1 PSUM pool closed -- they ride the late pool
                pool = mm
                if late_pool is not None and oc_i >= 6:
                    pool = late_pool["p"]
                psq = pool.tile([P, TOK], F32, tag="mm", name=f"psq{img}_{oc}")
                for hf in range(2):
                    sl = slice(hf * 512, (hf + 1) * 512)
                    for ci in range(NCC):
                        nc.tensor.matmul(
                            psq[:, sl],
                            lhsT=wqkvT[:, ci, oc * P:(oc + 1) * P],
                            rhs=xb[img][:, ci, sl],
                            start=(ci == 0), stop=False)
                    nc.tensor.matmul(
                        psq[:, sl],
                        lhsT=negwsum[0:1, oc * P:(oc + 1) * P],
                        rhs=mu16[img][0:1, sl],
                        start=False, stop=True)

                # padded tile: zero borders only, then fused evac * rstd
                qkvp = qp.tile([P, PTOK], F16, tag="qkvp", name=f"qv{img}_{oc}")
                nc.gpsimd.memset(qkvp[:, 0:34], 0.0)
                edge = qkvp[:, 33:33 + 33 * PW].rearrange(
                    "p (r c) -> p r c", c=PW)[:, :, 0:2]
                nc.gpsimd.memset(edge, 0.0)
                nc.gpsimd.memset(qkvp[:, 33 * PW:PTOK], 0.0)
                for hf in range(2):
                    sl = slice(hf * 512, (hf + 1) * 512)
                    nc.vector.tensor_tensor(
                        _pad_view(qkvp, PW + 1 + hf * 16 * PW, 16),
                        psq[:, sl].rearrange("p (x y) -> p x y", y=S),
                        rsbc[img][:, sl].rearrange("p (x y) -> p x y", y=S),
                        OP.mult)

                # --- depthwise taps (per-image placement, see TAP_CFG) ---
                cfg = TAP_CFG[img]
                # PE taps accumulate into the same PSUM tile (fresh group)
                for ti, (dx, dy) in enumerate(cfg["pe"]):
                    di = PE_TAPS_UNION.index((dx, dy))
                    for hf in range(2):
                        rhs = _pad_view(qkvp, _tap_off(dx, dy) + hf * 16 * PW, 16)
                        nc.tensor.matmul(
                            psq[:, hf * 512:(hf + 1) * 512],
                            lhsT=dwdiag[:, oc, di, :],
                            rhs=rhs,
                            start=(ti == 0), stop=(ti == len(cfg["pe"]) - 1))

                # scalar taps: per-partition scale Copy into own acc tiles
                sc_acc = []
                for si, (dx, dy) in enumerate(cfg["sc"]):
                    a = accp.tile([P, TOK], F16, tag="acc", name=f"sa{img}_{oc}_{si}")
                    nc.scalar.activation(
                        a[:].rearrange("p (x y) -> p x y", y=S),
                        _pad_view(qkvp, _tap_off(dx, dy), S),
                        AF.Copy, scale=dwW[:, oc, _tidx((dx, dy)):_tidx((dx, dy)) + 1])
                    sc_acc.append(a)

                yield
                if oc < 8:
                    dest = qk_sb[img][:, oc, :]
                    vtmp = None
                else:
                    vtmp = dwp.tile([P, TOK], F16, tag="vtmp", name=f"vt{img}_{oc}")
                    dest = vtmp[:]

                # DVE STT chain (first op reads the PSUM partial), then
                # TT merges of the scalar-engine taps; last op writes dest.
                # With every tap on PE, a single evacuation copy suffices.
                n_ops = len(cfg["ch"]) + len(sc_acc)
                if n_ops == 0:
                    nc.vector.tensor_copy(out=dest, in_=psq[:])
                k = 0
                prev = psq[:].rearrange("p (x y) -> p x y", y=S)
                for (dx, dy) in cfg["ch"]:
                    k += 1
                    if k == n_ops:
                        o = dest.rearrange("p (x y) -> p x y", y=S)
                    else:
                        a = accp.tile([P, TOK], F16, tag="acc",
                                      name=f"ch{img}_{oc}_{k}")
                        o = a[:].rearrange("p (x y) -> p x y", y=S)
                    nc.vector.scalar_tensor_tensor(
                        o, _pad_view(qkvp, _tap_off(dx, dy), S),
                        dwW[:, oc, _tidx((dx, dy)):_tidx((dx, dy)) + 1],
                        prev, OP.mult, OP.add)
                    prev = o
                prev_flat = prev.rearrange("p x y -> p (x y)")
                for acc, eng in zip(sc_acc, cfg["mg"]):
                    k += 1
                    o = dest if k == n_ops else accp.tile(
                        [P, TOK], F16, tag="acc", name=f"mg{img}_{oc}_{k}")[:]
                    e = nc.vector if eng == "v" else nc.gpsimd
                    e.tensor_tensor(o, prev_flat, acc[:], OP.add)
                    prev_flat = o

                # v chunks: transpose to (token, d) into augmented vhat
                if oc >= 8:
                    pr = oc - 8
                    for jc in range(NJC):
                        tt = ttp.tile([P, P], F16, tag="tt",
                                      name=f"tt{img}_{oc}_{jc}")
                        nc.sync.dma_start(
                            tt[:], vtmp[:, jc * P:(jc + 1) * P], transpose=True)
                        nc.gpsimd.tensor_copy(
                            out=vhat[img][:, jc, 2 * pr:2 * pr + 2, 0:64],
                            in_=tt[:].rearrange("p (h d) -> p h d", h=2))
                yield

        # ================= stage 2 generator =================
        def s2(img, sp, np_):
            steps = [(h, jc) for h in range(HEADS) for jc in range(NJC)]
            ebq = {}
            recs = {}

            def load_eb(t):
                h, jc = steps[t]
                eb = ep.tile([P, TOK], F16, tag="eb", name=f"eb{img}_{h}_{jc}")
                nc.sync.dma_start(eb[:], ebt_d[h, jc])
                ebq[t] = eb

            for t0 in range(5):
                load_eb(t0)
            av = None
            for t, (h, jc) in enumerate(steps):
                if t + 5 < len(steps):
                    load_eb(t + 5)
                eb = ebq.pop(t)
                oc_q = h // 2
                r0 = (h % 2) * 64
                if jc == 0:
                    av = avp.tile([65, TOK], F32, tag="av", name=f"av{img}_{h}")
                ps_sim = sp.tile([P, TOK], F32, tag="sim",
                                   name=f"sim{img}_{h}_{jc}")
                lhsT = qk_sb[img][r0:r0 + 64, 4 + oc_q, jc * P:(jc + 1) * P]
                for hf in range(2):
                    sl = slice(hf * 512, (hf + 1) * 512)
                    nc.tensor.matmul(ps_sim[:, sl], lhsT=lhsT,
                                     rhs=qk_sb[img][r0:r0 + 64, oc_q, sl],
                                     start=True, stop=True)
                E = Ep.tile([P, TOK], F16, tag="ee", name=f"ee{img}_{h}_{jc}")
                nc.scalar.activation(E[:], ps_sim[:], AF.Exp)
                nc.vector.tensor_tensor(E[:], E[:], eb[:], OP.mult)
                for hf in range(2):
                    sl = slice(hf * 512, (hf + 1) * 512)
                    nc.tensor.matmul(av[:, sl],
                                     lhsT=vhat[img][:, jc, h, :],
                                     rhs=E[:, sl],
                                     start=(jc == 0), stop=(jc == NJC - 1))
                if jc == NJC - 1:
                    # head output -> outT (DVE); softmax denominator ->
                    # approx reciprocal straight off PSUM, fp16 via gpsimd
                    nc.vector.tensor_copy(out=outT[img][r0:r0 + 64, oc_q, :],
                                          in_=av[0:64, :])
                    dn32 = rcd.tile([1, TOK], F32, tag="dn32",
                                    name=f"dn32{img}_{h}")
                    nc.scalar.activation(dn32[:], av[64:65, :], AF.Copy)
                    rc32 = rcp.tile([1, TOK], F32, tag="rc32",
                                    name=f"rc32{img}_{h}")
                    nc.vector.reciprocal_approx_fast(out=rc32[:], in_=dn32[:])
                    if DBG:
                        nc.sync.dma_start(dbgdn_d[img, h:h + 1, :], dn32[:])
                        nc.sync.dma_start(dbgrc_d[img, h:h + 1, :], rc32[:])
                    recs[h] = rc32
                    if h % 2 == 1:
                        pr = h // 2
                        ps_bc = np_.tile([P, TOK], F32, tag="mm",
                                        name=f"nb{img}_{pr}")
                        for hf in range(2):
                            sl = slice(hf * 512, (hf + 1) * 512)
                            nc.tensor.matmul(ps_bc[:, sl],
                                             lhsT=selA[:],
                                             rhs=recs[h - 1][:, sl],
                                             start=True, stop=False)
                            nc.tensor.matmul(ps_bc[:, sl],
                                             lhsT=selB[:],
                                             rhs=recs[h][:, sl],
                                             start=False, stop=True)
                        rb = Ep.tile([P, TOK], F16, tag="ee",
                                     name=f"rb{img}_{pr}")
                        nc.vector.tensor_copy(out=rb[:], in_=ps_bc[:])
                        nc.vector.tensor_tensor(outT[img][:, pr, :],
                                                outT[img][:, pr, :], rb[:],
                                                OP.mult)
                yield

        # ================= stage 3 generator (out projection) =========
        def s3(img, np_):
            for oc4 in range(NCC):
                ps_o = np_.tile([P, TOK], F32, tag="mm", name=f"pso{img}_{oc4}")
                for hf in range(2):
                    sl = slice(hf * 512, (hf + 1) * 512)
                    for kc in range(NCC):
                        nc.tensor.matmul(
                            ps_o[:, sl],
                            lhsT=woutT[:, kc, oc4 * P:(oc4 + 1) * P],
                            rhs=outT[img][:, kc, sl],
                            start=(kc == 0), stop=(kc == NCC - 1))
                of = ofp.tile([P, TOK], F16, tag="of", name=f"of{img}_{oc4}")
                if oc4 % 2 == 0:
                    nc.scalar.activation(of[:], ps_o[:], AF.Copy)
                else:
                    nc.vector.tensor_copy(out=of[:], in_=ps_o[:])
                nc.gpsimd.dma_start(out_d[img, oc4 * P:(oc4 + 1) * P, :], of[:])
                yield

        def interleave(main, aux, ratio):
            n = 0
            for _ in main:
                n += 1
                if n % ratio == 0:
                    next(aux, None)
            for _ in aux:
                pass

        # phase A: stage 1 of image 0
        for _ in s1(0):
            pass
        # phase B: attention(img0) with stage-1(img1) as PE filler
        interleave(s2(0, simB, mm), s1(1), 2)
        # stage-1 PSUM pools retire; phase C double-buffers the sim PSUM
        ab_ctx.close()
        simC = ctx.enter_context(tc.tile_pool(name="simC", bufs=2,
                                              space="PSUM"))
        s3ps = ctx.enter_context(tc.tile_pool(name="s3ps", bufs=1,
                                              space="PSUM"))
        # phase C: attention(img1) with out-proj(img0) as filler
        interleave(s2(1, simC, s3ps), s3(0, s3ps), 13)
        # phase D: normalize + out proj of image 1
        for _ in s3(1, s3ps):
            pass

        if DBG:
            for img in range(IPC):
                nc.sync.dma_start(dbgqk_d[img], qk_sb[img][:])
                nc.sync.dma_start(dbgvh_d[img], vhat[img][:])
                nc.sync.dma_start(dbgot_d[img], outT[img][:])
                nc.sync.dma_start(dbgrs_d[img], rsbc[img][:])

    return nc


# ------------------------- host side -------------------------

def _rel_pos_indices(size):
    ar = np.arange(size)
    pos = np.stack(np.meshgrid(ar, ar, indexing="ij"), axis=-1).reshape(-1, 2)
    rel = pos[:, None, :] - pos[None, :, :] + size - 1
    return rel[..., 0] * (2 * size - 1) + rel[..., 1]


_NC_CACHE = None


def _get_nc():
    global _NC_CACHE
    if _NC_CACHE is None:
        _NC_CACHE = build_nc()
        _NC_CACHE.finalize()
    return _NC_CACHE


def kernel(x, gamma, w_qkv, dw_w_q, dw_b_q, dw_w_k, dw_b_k, dw_w_v, dw_b_v,
           w_out, pos_emb):
    x = np.asarray(x, np.float32).reshape(B, C, TOK)
    gamma_c = np.asarray(gamma, np.float32).reshape(C)
    w_qkv = np.asarray(w_qkv, np.float32)
    w_out = np.asarray(w_out, np.float32)
    pos_emb = np.asarray(pos_emb, np.float32)

    # fold gamma into qkv weights; transpose to (c, o); chunk for SBUF layout
    w_eff = w_qkv * gamma_c[None, :]
    wqkvT = np.ascontiguousarray(
        w_eff.T.reshape(NCC, P, O3).transpose(1, 0, 2)).astype(np.float16)
    negwsum = (-w_eff.sum(axis=1))[None, :].astype(np.float16)
    woutT = np.ascontiguousarray(
        w_out.T.reshape(NCC, P, INNER).transpose(1, 0, 2)).astype(np.float16)

    # depthwise taps: (o, 9) in canonical TAPS order, q taps folded with scale
    dww = np.concatenate([
        np.asarray(dw_w_q, np.float32).reshape(INNER, 9) * SCALE,
        np.asarray(dw_w_k, np.float32).reshape(INNER, 9),
        np.asarray(dw_w_v, np.float32).reshape(INNER, 9)], axis=0)
    dwb = np.concatenate([
        np.asarray(dw_b_q, np.float32) * SCALE,
        np.asarray(dw_b_k, np.float32),
        np.asarray(dw_b_v, np.float32)], axis=0)
    assert np.all(dwb == 0.0), "nonzero dwconv bias not supported by this kernel"
    dwW = np.ascontiguousarray(
        dww.reshape(NOC, P, 9).transpose(1, 0, 2)).astype(np.float32)

    dwdiag = np.zeros((P, NOC, len(PE_TAPS_UNION), P), np.float32)
    for oc in range(NOC):
        for ti, tap in enumerate(PE_TAPS_UNION):
            col = TAPS.index(tap)
            for p in range(P):
                dwdiag[p, oc, ti, p] = dww[oc * P + p, col]
    dwdiag = dwdiag.astype(np.float16)

    # exp of transposed relative-position bias: ebt[h, jc, j_in_chunk, i]
    idx = _rel_pos_indices(S)                       # (TOK, TOK)
    bias = pos_emb[idx]                             # (i, j, h)
    ebt = np.exp(bias.transpose(2, 1, 0))           # (h, j, i)
    ebt = np.ascontiguousarray(
        ebt.reshape(HEADS, NJC, P, TOK)).astype(np.float16)

    selpair = np.zeros((2, P), np.float32)
    selpair[0, :64] = 1.0
    selpair[1, 64:] = 1.0

    shared = dict(wqkvT=wqkvT, negwsum=negwsum, woutT=woutT, dwW=dwW,
                  dwdiag=dwdiag, ebt=ebt, selpair=selpair)
    in_maps = [dict(x=np.ascontiguousarray(x[i * IPC:(i + 1) * IPC]), **shared)
               for i in range(NCORES)]

    global last_in_maps
    last_in_maps = in_maps
    res = run_bass_kernel_spmd(_get_nc(), in_maps, list(range(NCORES)))
    out = np.concatenate([r["out"] for r in res.results], axis=0)
    return out.reshape(B, C, S, S).astype(np.float32)
